# revision 49
# baseline (speedup 1.0000x reference)
# Multi-head causal attention (B=4, S=2048, D=1024, H=16) on 8 TRN2 NeuronCores.
#
# Sharding: batch x head-half. Core c handles batch b=c//2 and heads
# [8p, 8p+8) where p=c%2 (d-model slice [512p, 512p+512)). Every core runs
# the identical causal program: Q/K/V projections for its 8 heads over the
# full sequence, causal attention for all 4 query chunks of 512, and a
# PARTIAL output projection y_part = ctx_local @ woT[512p:512p+512] + b_o/2.
# The host unshards by summing the two partial outputs of each batch pair.
# No cross-core collectives; zero duplicated projection work; causal
# structure exploited exactly: on diagonal kk-tiles the score matmuls, exp
# activations, AV matmuls and triangle mask-muls are all column-trimmed to
# the valid causal window.
#
# Matmuls in bf16 (fp32 PSUM); softmax stats fp32. Attention uses
# transposed scores St[kk, q]:
#   St = Kt_tile.T @ Qt  (2 heads packed per PSUM tile via tile_position)
#   P = exp(St) (trimmed); triangle window of P *= tri mask in place
#   OT[dv, q] += V_aug[kk, 65].T @ P   (ones column -> denominators free)
#   OT_norm = OT * reciprocal(bcast(denoms))
# Projection / output-projection PSUM groups are pumped from a filler queue
# INSIDE the attention step loop (borrowing St-pool slots) so the PE stays
# busy while the Scalar engine works through the exps. Inputs are uploaded
# pre-blocked (SBUF layout order) so every load is one fully-contiguous
# DMA, spread across three queues.
import sys

if '/opt/trn_rl_repo' not in sys.path:
    sys.path.insert(0, '/opt/trn_rl_repo')

import numpy as np

B, S, D = 4, 2048, 1024
H, DK = 16, 64
NCORES = 8
SC = 512                 # query chunk
NHP = 4                  # local head-pairs per core (8 heads)
NCHUNK = S // SC         # 4 query chunks, chunk ci needs 4*(ci+1) kk tiles

_CACHE = {}


def _build_program():
    import contextlib

    import concourse.tile as tile
    from concourse import bacc, mybir

    F32 = mybir.dt.float32
    BF16 = mybir.dt.bfloat16
    EXP = mybir.ActivationFunctionType.Exp

    nc = bacc.Bacc("TRN2", target_bir_lowering=False, debug=False,
                   num_devices=NCORES)

    FP8 = mybir.dt.float8e4
    DR = mybir.MatmulPerfMode.DoubleRow

    # all inputs pre-blocked on host into SBUF layout (contiguous DMAs).
    # x8/wk8/wq8 are fp8e4m3 with ctile PAIRS packed for DoubleRow matmuls
    # (contraction 256 per instruction -> half the projection matmul count)
    xT_d = nc.dram_tensor("xT", [4, 128, 8 * 512], BF16,
                          kind="ExternalInput")
    x8_d = nc.dram_tensor("x8", [4, 128, 8 * 512], FP8,
                          kind="ExternalInput")
    wq_d = nc.dram_tensor("wq", [128, 8 * 512], FP8, kind="ExternalInput")
    wk_d = nc.dram_tensor("wk", [128, 8 * 512], FP8, kind="ExternalInput")
    wv_d = nc.dram_tensor("wv", [128, 8 * 512], BF16, kind="ExternalInput")
    wo_d = nc.dram_tensor("wo", [128, 4 * 1024], BF16, kind="ExternalInput")
    bias_d = nc.dram_tensor("bias", [1, D], BF16, kind="ExternalInput")
    masks_d = nc.dram_tensor("masks", [128, 4 * 1024], BF16,
                             kind="ExternalInput")
    y_d = nc.dram_tensor("y", [S, D], F32, kind="ExternalOutput")

    with tile.TileContext(nc) as tc, contextlib.ExitStack() as ctx:
        smalls = ctx.enter_context(tc.tile_pool(name="smalls", bufs=1))
        p_OT = ctx.enter_context(tc.tile_pool(name="otp", bufs=1))
        p_Kt = ctx.enter_context(tc.tile_pool(name="ktp", bufs=1))
        p_Qt = ctx.enter_context(tc.tile_pool(name="qtp", bufs=1))
        p_V = ctx.enter_context(tc.tile_pool(name="vp", bufs=1))
        p_mk = ctx.enter_context(tc.tile_pool(name="mk", bufs=1))
        p_w = ctx.enter_context(tc.tile_pool(name="wp", bufs=1))
        p_x = ctx.enter_context(tc.tile_pool(name="xp", bufs=4))
        p_rs = ctx.enter_context(tc.tile_pool(name="rs", bufs=4))
        p_avs = ctx.enter_context(tc.tile_pool(name="avs", bufs=8))
        p_bc = ctx.enter_context(tc.tile_pool(name="bcp", bufs=2))
        p_P = ctx.enter_context(tc.tile_pool(name="pp", bufs=8))
        p_yb = ctx.enter_context(tc.tile_pool(name="ybp", bufs=2))
        p_st = ctx.enter_context(tc.tile_pool(name="pst", bufs=2,
                                              space="PSUM"))
        p_av = ctx.enter_context(tc.tile_pool(name="pav", bufs=4,
                                              space="PSUM"))

        # ---------------- weights / constants / x DMA --------------------
        # wk/wq (fp8, DoubleRow pair layout) are blocked by head-pair
        # halves so the first attention block's weights arrive first
        wk = p_w.tile([128, 8 * 512], FP8, tag="wk")
        wq = p_w.tile([128, 8 * 512], FP8, tag="wq")
        wv = p_w.tile([128, 8 * 512], BF16, tag="wv")
        wo = p_w.tile([128, 4 * 1024], BF16, tag="wo")
        nc.scalar.dma_start(wk[:, 0:2048], wk_d.ap()[:, 0:2048])
        nc.scalar.dma_start(wq[:, 0:2048], wq_d.ap()[:, 0:2048])
        nc.scalar.dma_start(wk[:, 2048:4096], wk_d.ap()[:, 2048:4096])
        nc.scalar.dma_start(wq[:, 2048:4096], wq_d.ap()[:, 2048:4096])

        xchs = []
        x8chs = []
        for sc in range(4):
            xchs.append(p_x.tile([128, 8 * 512], BF16, tag="xch",
                                 name=f"xch_{sc}"))
            x8chs.append(p_x.tile([128, 8 * 512], FP8, tag="x8ch",
                                  name=f"x8ch_{sc}"))
        # wv behind the 1MB of fp8 weights on the scalar queue beats
        # queueing it behind 1.5MB of x data on the sync queue
        nc.scalar.dma_start(wv[:], wv_d.ap())
        nc.sync.dma_start(x8chs[0][:], x8_d.ap()[0])
        nc.sync.dma_start(xchs[0][:], xT_d.ap()[0])
        for sc in range(1, 4):
            nc.sync.dma_start(x8chs[sc][:], x8_d.ap()[sc])
            nc.sync.dma_start(xchs[sc][:], xT_d.ap()[sc])

        masks_sb = p_mk.tile([128, 4 * 1024], BF16, tag="masks")
        nc.gpsimd.dma_start(masks_sb[:], masks_d.ap())
        nc.gpsimd.dma_start(wo[:], wo_d.ap())
        bias_sb = smalls.tile([1, D], BF16, tag="bias")
        nc.gpsimd.dma_start(bias_sb[:], bias_d.ap())
        biasbc = smalls.tile([128, D], BF16, tag="biasbc")
        nc.gpsimd.partition_broadcast(biasbc[:], bias_sb[:])

        onesf = smalls.tile([128, 128], F32, tag="onesf")
        nc.vector.memset(onesf[:], 1.0)

        OT = p_OT.tile([128, NHP * S], BF16, tag="OT")
        Kt = p_Kt.tile([128, NHP * S], BF16, tag="Kt")
        Qt = p_Qt.tile([128, NHP * S], BF16, tag="Qt")
        Vsb = p_V.tile([128, 16 * 8 * 65], BF16, tag="Vsb")

        # ones columns of V_aug (16 s-tiles x 8 heads, one strided copy)
        nc.vector.tensor_copy(
            Vsb[:].rearrange("p (s h c) -> p s h c", s=16, c=65)
            [:, :, :, 64:65],
            onesf[:].rearrange("p (s h) -> p s h", s=16)[:, :, :, None])

        # ---------------- projection / out-proj group emitters -----------
        # each closure emits one PSUM group; it borrows a slot from either
        # the p_av pool ([128,512] = 1 bank) or the p_st pool ([128,1024]
        # slot, first half used)
        def alloc_ps(ps_pool, name):
            if ps_pool is p_st:
                t = ps_pool.tile([128, 1024], F32, tag="st", name=name)
                return t[:, 0:512]
            return ps_pool.tile([128, 512], F32, tag="av", name=name)

        def k_group(sc, hp, kq, dst):
            w_t = wk if kq == 'k' else wq

            def emit(ps_pool):
                ps = alloc_ps(ps_pool, f"ps{kq}_{sc}_{hp}")
                half, lane = hp // 2, hp % 2
                for j in range(4):
                    off = half * 2048 + j * 512 + lane * 256
                    nc.tensor.matmul(
                        ps[:],
                        w_t[:, off:off + 256]
                        .rearrange("p (i m) -> p i m", i=2),
                        x8chs[sc][:, j * 1024:(j + 1) * 1024]
                        .rearrange("p (i s) -> p i s", i=2),
                        start=(j == 0), stop=(j == 3),
                        perf_mode=DR)
                nc.vector.tensor_copy(
                    dst[:, hp * S + sc * 512:hp * S + (sc + 1) * 512], ps[:])
            return emit

        def v_group(st_g):
            def emit(ps_pool):
                ps = alloc_ps(ps_pool, f"psv_{st_g}")
                sti = st_g % 4
                for k in range(8):
                    nc.tensor.matmul(
                        ps[:],
                        xchs[st_g // 4][:, k * 512 + sti * 128:
                                        k * 512 + (sti + 1) * 128],
                        wv[:, k * 512:(k + 1) * 512],
                        start=(k == 0), stop=(k == 7))
                nc.vector.tensor_copy(
                    Vsb[:, st_g * 520:(st_g + 1) * 520]
                    .rearrange("p (h c) -> p h c", c=65)[:, :, 0:64],
                    ps[:].rearrange("p (h c) -> p h c", c=64))
            return emit

        def y_group(ci, qi, nc2):
            def emit(ps_pool):
                ps = alloc_ps(ps_pool, f"psy_{ci}_{qi}_{nc2}")
                for dc in range(4):
                    nc.tensor.matmul(
                        ps[:],
                        OT[:, dc * S + ci * SC + qi * 128:
                           dc * S + ci * SC + (qi + 1) * 128],
                        wo[:, dc * 1024 + nc2 * 512:
                           dc * 1024 + (nc2 + 1) * 512],
                        start=(dc == 0), stop=(dc == 3))
                yb = p_yb.tile([128, 512], F32, tag="yb")
                nc.vector.tensor_add(
                    yb[:], ps[:],
                    biasbc[:, nc2 * 512:(nc2 + 1) * 512])
                eng = nc.sync if (qi + nc2) % 2 == 0 else nc.gpsimd
                eng.dma_start(
                    y_d.ap()[ci * SC + qi * 128:ci * SC + (qi + 1) * 128,
                             nc2 * 512:(nc2 + 1) * 512], yb[:])
            return emit

        def stage_groups(sc, first=False):
            gs = []
            if first:
                # ordered by DMA arrival: wk/wq first halves land first,
                # then wv, then the second weight halves
                for hp in (0, 1):
                    gs.append(k_group(sc, hp, 'k', Kt))
                for hp in (0, 1):
                    gs.append(k_group(sc, hp, 'q', Qt))
                for sti in range(4):
                    gs.append(v_group(sti))
                for hp in (2, 3):
                    gs.append(k_group(sc, hp, 'k', Kt))
                    gs.append(k_group(sc, hp, 'q', Qt))
                return gs
            for hp in range(NHP):
                gs.append(k_group(sc, hp, 'k', Kt))
                gs.append(k_group(sc, hp, 'q', Qt))
            for sti in range(4):
                gs.append(v_group(4 * sc + sti))
            return gs

        # filler queue: (deadline_stage, closure). Groups with
        # deadline_stage <= s must be flushed before chunk s starts.
        # normalize chains (nq) drain with priority over filler groups --
        # this also guarantees out-proj groups are emitted after their
        # chunk's OT writes.
        filler = []
        nq = []

        def pump(n, pool):
            for _ in range(n):
                if nq:
                    nq.pop(0)()
                elif filler:
                    _, emit = filler.pop(0)
                    emit(pool)
                else:
                    return

        def flush_nq():
            while nq:
                nq.pop(0)()

        def flush_stage(s, pool):
            keep = []
            for dl, emit in filler:
                if dl <= s:
                    emit(pool)
                else:
                    keep.append((dl, emit))
            filler[:] = keep

        # stage 0 runs upfront (attention depends on it)
        for g in stage_groups(0, first=True):
            g(p_av)
        for dl, sc in ((1, 1), (2, 2), (3, 3)):
            filler.extend((dl, g) for g in stage_groups(sc))

        # ---------------- attention ----------------
        for ci in range(NCHUNK):
            cap = 4 * (ci + 1)
            for bl in range(2):
                av = [p_av.tile([128, 512], F32, tag="av",
                                name=f"av_{ci}_{bl}_{i}")
                      for i in range(4)]

                def emit_av(t, p_tiles, cap=cap, av=av, bl=bl, ci=ci):
                    r = t - 4 * ci
                    q0 = 128 * r if r >= 0 else 0
                    for hp_i in range(2):
                        for hh in range(2):
                            hi = 2 * hp_i + hh
                            h = (2 * bl + hp_i) * 2 + hh
                            off = t * 520 + h * 65
                            nc.tensor.matmul(
                                av[hi][0:65, q0:512],
                                Vsb[:, off:off + 65],
                                p_tiles[hp_i][:, hh * 512 + q0:
                                              (hh + 1) * 512],
                                start=(t == 0), stop=(t == cap - 1),
                                skip_group_check=(r >= 0))

                pending = []
                for t in range(cap):
                    r = t - 4 * ci  # >=0 -> diagonal tile
                    q0 = 128 * r if r >= 0 else 0
                    p_cur = []
                    for hp_i in range(2):
                        hp = 2 * bl + hp_i
                        st = p_st.tile([128, 1024], F32, tag="st")
                        for hh in range(2):
                            r0 = 64 * hh
                            nc.tensor.matmul(
                                st[:, hh * 512 + q0:(hh + 1) * 512],
                                Kt[r0:r0 + 64,
                                   hp * S + t * 128:hp * S + (t + 1) * 128],
                                Qt[r0:r0 + 64,
                                   hp * S + ci * SC + q0:
                                   hp * S + (ci + 1) * SC],
                                start=True, stop=True,
                                tile_position=(r0, 0))
                        # wk/wq are uploaded x64 for fp8 range; fold the
                        # compensation and the 1/sqrt(dk) softmax scale
                        # into the exp: exp(St/(64*64*8))
                        ESC = 1.0 / 32768.0
                        p1 = p_P.tile([128, 1024], BF16, tag="p")
                        if r >= 1:
                            nc.scalar.activation(
                                p1[:].rearrange("p (h q) -> p h q", h=2)
                                [:, :, q0:512],
                                st[:].rearrange("p (h q) -> p h q", h=2)
                                [:, :, q0:512],
                                EXP, scale=ESC)
                        else:
                            nc.scalar.activation(p1[:], st[:], EXP,
                                                 scale=ESC)
                        if r >= 0:
                            # triangle mask on the diagonal 128-col window
                            nc.vector.tensor_mul(
                                p1[:].rearrange("p (h q) -> p h q", h=2)
                                [:, :, q0:q0 + 128],
                                p1[:].rearrange("p (h q) -> p h q", h=2)
                                [:, :, q0:q0 + 128],
                                masks_sb[:, r * 1024:(r + 1) * 1024]
                                .rearrange("p (h q) -> p h q", h=2)
                                [:, :, q0:q0 + 128])
                        p_cur.append(p1)
                    pending.append((t, p_cur))
                    if len(pending) > 2:
                        tt, pp_t = pending.pop(0)
                        emit_av(tt, pp_t)
                    pump(1, p_st)
                for tt, pp_t in pending:
                    emit_av(tt, pp_t)

                # free the av PSUM banks fast: copy raw accumulators (and
                # their denominator row 64) to SBUF, alternating between
                # the Scalar and Vector engines; the actual normalization
                # is deferred into the next block's step loop via nq
                avs = []
                for hi in range(4):
                    t_avs = p_avs.tile([65, 512], F32, tag="avs",
                                       name=f"avs_{ci}_{bl}_{hi}")
                    if hi % 2 == 0:
                        nc.scalar.activation(
                            t_avs[:], av[hi][0:65, :],
                            mybir.ActivationFunctionType.Copy)
                    else:
                        nc.vector.tensor_copy(t_avs[:], av[hi][0:65, :])
                    avs.append(t_avs)

                def norm_chain(hp_i, hh, bl=bl, ci=ci, avs=avs):
                    hp = 2 * bl + hp_i
                    hi = 2 * hp_i + hh
                    r0 = 64 * hh
                    # denominator row to partition 0 (partition_broadcast
                    # reads partition 0 only); tiny SBUF->SBUF DMA off
                    # both compute engines
                    rs = p_rs.tile([1, 512], F32, tag="rs")
                    nc.gpsimd.dma_start(rs[:], avs[hi][64:65, :])
                    bc = p_bc.tile([64, 512], F32, tag="bc")
                    nc.gpsimd.partition_broadcast(bc[:], rs[:])
                    rbc = p_bc.tile([64, 512], F32, tag="rbc")
                    scr = p_bc.tile([64, 512], F32, tag="scr")
                    nc.vector.reciprocal_approx_accurate(
                        rbc[:], bc[:], scratch=scr[:])
                    nc.vector.tensor_mul(
                        OT[r0:r0 + 64,
                           hp * S + ci * SC:hp * S + (ci + 1) * SC],
                        avs[hi][0:64, :],
                        rbc[:])

                for hp_i in range(2):
                    for hh in range(2):
                        nq.append(lambda h=hp_i, j=hh, f=norm_chain: f(h, j))

            # next chunk needs its K/Q/V projections complete
            flush_stage(ci + 1, p_st)
            # out-proj of this chunk becomes deadline-free filler work; the
            # nq-priority in pump() guarantees its normalize chains are
            # emitted first
            for qi in range(4):
                for nc2 in range(2):
                    filler.append((99, y_group(ci, qi, nc2)))
        flush_nq()
        flush_stage(99, p_st)

    nc.compile()
    return nc


def _get_program():
    if 'nc' not in _CACHE:
        _CACHE['nc'] = _build_program()
    return _CACHE['nc']


def _tri_masks():
    # masks[r] = [128, 2 x 512]: per hh half, [zeros(128r) | tri | ones]
    # (only the 128-wide triangle window is read by the mask-muls)
    import ml_dtypes
    p = np.arange(128)[:, None]
    f = np.arange(512)[None, :]
    out = np.zeros((128, 4 * 1024), np.float32)
    for r in range(4):
        m = (p <= f - 128 * r).astype(np.float32)  # valid: key<=query
        out[:, r * 1024:r * 1024 + 512] = m
        out[:, r * 1024 + 512:(r + 1) * 1024] = m
    return out.astype(ml_dtypes.bfloat16)


def kernel(x, w_q, w_k, w_v, w_o, b_o):
    import ml_dtypes
    from concourse.bass_utils import run_bass_kernel_spmd

    BF = ml_dtypes.bfloat16
    x = np.asarray(x, dtype=np.float32)
    nc = _get_program()

    # wq/wk go to fp8e4m3: upload x64 so the uniform(-1/32,1/32) weights
    # sit in fp8's normal range; the exp activation divides it back out
    # (together with the 1/sqrt(dk) softmax scale)
    wqT = np.ascontiguousarray(np.asarray(w_q, np.float32).T * 64.0)
    wkT = np.ascontiguousarray(np.asarray(w_k, np.float32).T * 64.0)
    wvT = np.ascontiguousarray(np.asarray(w_v, np.float32).T)
    woT = np.ascontiguousarray(np.asarray(w_o, np.float32).T)
    bias_half = (np.asarray(b_o, np.float32) * 0.5)[None, :]
    masks = _tri_masks()

    def blk_w(w):   # [1024, 512] -> [128, 8*512] in SBUF layout
        return np.ascontiguousarray(
            w.reshape(8, 128, 512).transpose(1, 0, 2).reshape(128, 4096)
        ).astype(BF)

    def blk_w_kq(w):
        # [1024, 512] -> [128, half x j x lane x i x 128] fp8, where the
        # contraction ctile = 2j+i is the DoubleRow pair index
        F8 = ml_dtypes.float8_e4m3
        return np.ascontiguousarray(
            w.reshape(4, 2, 128, 2, 2, 128).transpose(2, 3, 0, 4, 1, 5)
            .reshape(128, 4096)).astype(F8)

    def blk_x8(xb):  # [2048, 1024] -> [4, 128, j x i x 512] fp8
        F8 = ml_dtypes.float8_e4m3
        xT = xb.T
        return np.ascontiguousarray(
            xT.reshape(4, 2, 128, 4, 512).transpose(3, 2, 0, 1, 4)
            .reshape(4, 128, 4096)).astype(F8)

    def blk_wo(w):  # [512, 1024] -> [128, 4*1024]
        return np.ascontiguousarray(
            w.reshape(4, 128, 1024).transpose(1, 0, 2).reshape(128, 4096)
        ).astype(BF)

    def blk_x(xb):  # [2048, 1024] -> [4, 128, 8*512]
        xT = xb.T  # [1024, 2048]
        return np.ascontiguousarray(
            xT.reshape(8, 128, 4, 512).transpose(2, 1, 0, 3)
            .reshape(4, 128, 4096)).astype(BF)

    xTs = [blk_x(x[b]) for b in range(B)]
    x8s = [blk_x8(x[b]) for b in range(B)]
    in_maps = []
    for c in range(NCORES):
        b, p = c // 2, c % 2
        sl = slice(p * 512, (p + 1) * 512)
        in_maps.append({
            "xT": xTs[b],
            "x8": x8s[b],
            "wq": blk_w_kq(wqT[:, sl]),
            "wk": blk_w_kq(wkT[:, sl]),
            "wv": blk_w(wvT[:, sl]),
            "wo": blk_wo(np.ascontiguousarray(woT[sl, :])),
            "bias": bias_half.astype(BF),
            "masks": masks,
        })

    res = run_bass_kernel_spmd(nc, in_maps, core_ids=list(range(NCORES)),
                               trace=_CACHE.get('trace', False),
                               tmpdir=_CACHE.get('tmpdir'))
    _CACHE['last_res'] = res

    y = np.empty((B, S, D), dtype=np.float32)
    for b in range(B):
        y[b] = res.results[2 * b]["y"] + res.results[2 * b + 1]["y"]
    return y
